# revision 1
# baseline (speedup 1.0000x reference)
"""NTM scatter-memory kernel for 8 Trainium2 NeuronCores (Bass/Tile).

Sharding: the [8192, 4096] memory is row-sharded across 8 cores; each
core's 1024x4096 shard lives in SBUF for all 8 steps (the final memory is
never returned, so there is no HBM traffic for it inside the loop).

Per step:
  - controller / write-key / erase / add vectors are computed on TensorE
    with the controller vector replicated across all 128 output partitions
    (stride-0 lhsT), so the nonlinearity lands directly on broadcast tiles.
  - content-addressing logits z = mem @ k and row norms are fused DVE
    scalar_tensor_tensor / ScalarE activation(accum_out) passes.
  - global softmax over 8192 slots is flash-style: AllGather of per-core
    (max, sum), local exp with global stats.
  - the rank-1 erase/add write is done in place on the SBUF shard.
  - read vector: TensorE weighted row-sum -> per-core partial read, scaled
    by the flash combine weight, AllReduce(add) -> full read everywhere.

Self-contained: shapes hardcoded; host prep in numpy.
"""

import numpy as np

M_SLOTS = 8192
N_DIM = 4096
FVS = 64
PLEN = 64
CDIM = 256
NIN, NOUT = 512, 512
NSTEPS = 8
EPS = 1e-8

N_CORES = 8
M_LOC = M_SLOTS // N_CORES          # 1024 rows per core
RT = M_LOC // 128                   # 8 row-tiles per core
NCH = N_DIM // 512                  # 8 column chunks of 512

MEM_BF16 = False                    # memory shard dtype (False -> fp32)

_CACHE = {}


def build_nc(steps=NSTEPS, mem_bf16=MEM_BF16):
    import concourse.bacc as bacc
    import concourse.mybir as mybir
    import concourse.tile as tile
    from concourse.bass_isa import ReduceOp

    F32 = mybir.dt.float32
    BF16 = mybir.dt.bfloat16
    MDT = BF16 if mem_bf16 else F32
    AL = mybir.AluOpType
    ACT = mybir.ActivationFunctionType
    AX = mybir.AxisListType

    try:
        import concourse.tile_utils as tile_utils
        tile_utils.max_sbuf_usage = 208 * 1024
    except Exception:
        pass

    nc = bacc.Bacc("TRN2", target_bir_lowering=False, debug=False,
                   num_devices=N_CORES)

    d_mem = nc.dram_tensor("mem", [128, RT * N_DIM], MDT, kind="ExternalInput")
    d_sqrtn0 = nc.dram_tensor("sqrtn0", [128, RT], F32, kind="ExternalInput")
    d_x0 = nc.dram_tensor("x0col", [FVS, 1], F32, kind="ExternalInput")
    d_prog = nc.dram_tensor("progpad", [128, NSTEPS], F32, kind="ExternalInput")
    d_wct = nc.dram_tensor("wct", [128, CDIM], F32, kind="ExternalInput")
    d_bc = nc.dram_tensor("bccol", [128, 2], F32, kind="ExternalInput")
    d_wt = nc.dram_tensor("wt", [CDIM, 3 * N_DIM], BF16, kind="ExternalInput")
    d_wtb = nc.dram_tensor("wtb", [1, 3 * N_DIM], BF16, kind="ExternalInput")
    d_kr = nc.dram_tensor("krall", [NSTEPS, N_DIM], BF16, kind="ExternalInput")
    d_oe = nc.dram_tensor("oesb", [FVS, NOUT], F32, kind="ExternalInput")
    d_ones = nc.dram_tensor("onesrow", [1, 128], BF16, kind="ExternalInput")
    d_out = nc.dram_tensor("out", [1, NOUT], F32, kind="ExternalOutput")

    RG = [list(range(N_CORES))]

    with tile.TileContext(nc) as tc:
        with (
            tc.tile_pool(name="pmem", bufs=1) as pmem,
            tc.tile_pool(name="pconst", bufs=1) as pconst,
            tc.tile_pool(name="pstate", bufs=2) as pstate,
            tc.tile_pool(name="pvb", bufs=3) as pvb,
            tc.tile_pool(name="ps1", bufs=2) as ps1,
            tc.tile_pool(name="pwt", bufs=4) as pwt,
            tc.tile_pool(name="psm", bufs=4) as psm,
            tc.tile_pool(name="prow", bufs=2) as prow,
            tc.tile_pool(name="prp", bufs=2) as prp,
            tc.tile_pool(name="pkr", bufs=1) as pkr,
            tc.tile_pool(name="pps", bufs=2, space="PSUM") as pps,
            tc.tile_pool(name="ppsb", bufs=2, space="PSUM") as ppsb,
            tc.tile_pool(name="ppsc", bufs=1, space="PSUM") as ppsc,
            tc.tile_pool(name="pdram", bufs=4, space="DRAM") as pdram,
        ):
            # ---- persistent state ----
            mem = pmem.tile([128, RT * N_DIM], MDT, tag="mem")
            nc.sync.dma_start(mem[:], d_mem[:])
            sqrtn = pstate.tile([128, RT], F32, tag="sqrtn")
            nc.sync.dma_start(sqrtn[:], d_sqrtn0[:])
            x_col = pstate.tile([FVS, 1], F32, tag="xcol")
            nc.sync.dma_start(x_col[:], d_x0[:])

            prog = pconst.tile([128, NSTEPS], F32, tag="prog")
            nc.sync.dma_start(prog[:], d_prog[:])
            wct = pconst.tile([128, CDIM], F32, tag="wct")
            nc.sync.dma_start(wct[:], d_wct[:])
            bccol = pconst.tile([128, 2], F32, tag="bccol")
            nc.sync.dma_start(bccol[:], d_bc[:])
            oesb = pconst.tile([FVS, NOUT], F32, tag="oesb")
            nc.sync.dma_start(oesb[:], d_oe[:])
            onesb = pconst.tile([1, 128], BF16, tag="onesb")
            nc.sync.dma_start(onesb[:], d_ones[:])

            def msl(rt):
                return slice(rt * N_DIM, (rt + 1) * N_DIM)

            # small-op helpers -------------------------------------------
            def neg_of(ap, tag):
                t = psm.tile([ap.shape[0], 1], F32, tag=tag)
                nc.vector.tensor_scalar(t[:], ap, -1.0, None, AL.mult)
                return t

            def colstats(li, tagp):
                """li [128, RT] -> (lmax[128,1], lsum[128,1], ex[128,RT])"""
                rmax = psm.tile([128, 1], F32, tag=tagp + "rmax")
                nc.vector.tensor_reduce(rmax[:], li, AX.X, AL.max)
                lmax = psm.tile([128, 1], F32, tag=tagp + "lmax")
                nc.gpsimd.partition_all_reduce(lmax[:], rmax[:], 128,
                                               ReduceOp.max)
                nlm = neg_of(lmax[:], tagp + "neg")
                ex = psm.tile([128, RT], F32, tag=tagp + "ex")
                nc.scalar.activation(ex[:], li, ACT.Exp, bias=nlm[:])
                rsum = psm.tile([128, 1], F32, tag=tagp + "rsum")
                nc.vector.tensor_reduce(rsum[:], ex[:], AX.X, AL.add)
                lsum = psm.tile([128, 1], F32, tag=tagp + "lsum")
                nc.gpsimd.partition_all_reduce(lsum[:], rsum[:], 128,
                                               ReduceOp.add)
                return lmax, lsum, ex

            for t in range(steps):
                # ---------- controller ----------
                cat = psm.tile([128, 1], F32, tag="cat")
                nc.vector.tensor_copy(cat[FVS:128, :], prog[FVS:128, t:t + 1])
                nc.vector.tensor_copy(cat[0:FVS, :], x_col[:])
                c_ps = ppsc.tile([128, 2], F32, tag="mini")
                nc.tensor.matmul(c_ps[:, 0:1], wct[:, 0:128], cat[:],
                                 start=True, stop=True)
                nc.tensor.matmul(c_ps[:, 1:2], wct[:, 128:256], cat[:],
                                 start=True, stop=True)
                c_sb = psm.tile([128, 2], BF16, tag="c_sb")
                for h in range(2):
                    nc.scalar.activation(c_sb[:, h:h + 1], c_ps[:, h:h + 1],
                                         ACT.Sigmoid, bias=bccol[:, h:h + 1])

                # ---------- k / e / a fused with broadcast ----------
                c0b = c_sb[:, 0:1].broadcast_to([128, 128])
                c1b = c_sb[:, 1:2].broadcast_to([128, 128])
                kea = []
                for m, func in ((0, ACT.Tanh), (1, ACT.Sigmoid), (2, ACT.Tanh)):
                    vb = pvb.tile([128, N_DIM], BF16, tag="vb")
                    for ch in range(NCH):
                        cbase = m * N_DIM + ch * 512
                        w0 = pwt.tile([128, 512], BF16, tag="wtc")
                        nc.sync.dma_start(w0[:], d_wt[0:128, cbase:cbase + 512])
                        w1 = pwt.tile([128, 512], BF16, tag="wtc")
                        nc.sync.dma_start(w1[:], d_wt[128:256, cbase:cbase + 512])
                        bc_ps = pps.tile([128, 512], F32, tag="bc_ps")
                        nc.tensor.matmul(bc_ps[:], c0b, w0[:],
                                         start=True, stop=False)
                        nc.tensor.matmul(bc_ps[:], c1b, w1[:],
                                         start=False, stop=False)
                        wb = pwt.tile([1, 512], BF16, tag="wtb")
                        nc.sync.dma_start(wb[:], d_wtb[0:1, cbase:cbase + 512])
                        nc.tensor.matmul(bc_ps[:], onesb[:], wb[:],
                                         start=False, stop=True)
                        nc.scalar.activation(vb[:, ch * 512:(ch + 1) * 512],
                                             bc_ps[:], func)
                    kea.append(vb)
                k_b, e_b, a_b = kea

                # ---------- ||k||^2 (every lane ends up with the value) ----
                dumb = psm.tile([128, 1], F32, tag="dumb")
                kk2 = psm.tile([128, 1], F32, tag="kk2")
                nc.vector.scalar_tensor_tensor(
                    dumb[:].broadcast_to([128, N_DIM]), k_b[:], 1.0, k_b[:],
                    AL.mult, AL.mult, accum_out=kk2[:])

                # ---------- z_w = mem @ k ----------
                zw = psm.tile([128, RT], F32, tag="zw")
                for rt in range(RT):
                    dz = psm.tile([128, 1], F32, tag="dz")
                    nc.vector.scalar_tensor_tensor(
                        dz[:].broadcast_to([128, N_DIM]), mem[:, msl(rt)], 1.0,
                        k_b[:], AL.mult, AL.mult,
                        accum_out=zw[:, rt:rt + 1])

                # ---------- write logits + local stats ----------
                kk = psm.tile([128, 1], F32, tag="kk")
                nc.scalar.activation(kk[:], kk2[:], ACT.Sqrt)
                den = psm.tile([128, RT], F32, tag="den")
                nc.vector.tensor_scalar(den[:], sqrtn[:], kk[:], EPS,
                                        AL.mult, AL.add)
                rec = psm.tile([128, RT], F32, tag="rec")
                nc.vector.reciprocal(rec[:], den[:])
                li_w = psm.tile([128, RT], F32, tag="li_w")
                nc.vector.tensor_tensor(li_w[:], zw[:], rec[:], AL.mult)
                lmax_w, lsum_w, _ = colstats(li_w[:], "w")

                # ---------- AllGather write stats ----------
                pay_a = prow.tile([1, 2], F32, tag="pay_a")
                nc.vector.tensor_copy(pay_a[0:1, 0:1], lmax_w[0:1, :])
                nc.vector.tensor_copy(pay_a[0:1, 1:2], lsum_w[0:1, :])
                ag_a_in = pdram.tile([1, 2], F32, tag="ag_a_in")
                ag_a_out = pdram.tile([N_CORES, 2], F32, tag="ag_a_out")
                nc.sync.dma_start(ag_a_in[:], pay_a[:])
                nc.gpsimd.collective_compute(
                    "AllGather", AL.bypass, replica_groups=RG,
                    ins=[ag_a_in.opt()], outs=[ag_a_out.opt()])
                st8 = psm.tile([N_CORES, 2], F32, tag="st8")
                nc.sync.dma_start(st8[:], ag_a_out[:])

                gm8 = psm.tile([N_CORES, 1], F32, tag="gm8")
                nc.gpsimd.partition_all_reduce(gm8[:], st8[:, 0:1], N_CORES,
                                               ReduceOp.max)
                ng8 = psm.tile([N_CORES, 1], F32, tag="ng8")
                nc.vector.tensor_scalar(ng8[:], gm8[:], -1.0, None, AL.mult)
                eh8 = psm.tile([N_CORES, 1], F32, tag="eh8")
                nc.scalar.activation(eh8[:], st8[:, 0:1], ACT.Exp, bias=ng8[:])
                pr8 = psm.tile([N_CORES, 1], F32, tag="pr8")
                nc.vector.tensor_tensor(pr8[:], eh8[:], st8[:, 1:2], AL.mult)
                gs8 = psm.tile([N_CORES, 1], F32, tag="gs8")
                nc.gpsimd.partition_all_reduce(gs8[:], pr8[:], N_CORES,
                                               ReduceOp.add)
                grow = prow.tile([1, 2], F32, tag="grow")
                nc.vector.tensor_copy(grow[0:1, 0:1], gm8[0:1, :])
                nc.vector.tensor_copy(grow[0:1, 1:2], gs8[0:1, :])
                gb = psm.tile([128, 2], F32, tag="gb")
                nc.gpsimd.partition_broadcast(gb[:], grow[:])
                ginv = psm.tile([128, 1], F32, tag="ginv")
                nc.vector.reciprocal(ginv[:], gb[:, 1:2])
                ngm = neg_of(gb[:, 0:1], "ngm")
                wex = psm.tile([128, RT], F32, tag="wex")
                nc.scalar.activation(wex[:], li_w[:], ACT.Exp, bias=ngm[:])
                w_col = psm.tile([128, RT], F32, tag="w_col")
                nc.vector.tensor_scalar(w_col[:], wex[:], ginv[:], None,
                                        AL.mult)

                # ---------- kr broadcast ----------
                krrow = pkr.tile([1, N_DIM], BF16, tag="krrow")
                nc.sync.dma_start(krrow[:], d_kr[t:t + 1, :])
                kr_b = pvb.tile([128, N_DIM], BF16, tag="vb")
                for ch in range(NCH):
                    kr_ps = pps.tile([128, 512], F32, tag="bc_ps")
                    nc.tensor.matmul(kr_ps[:], onesb[:],
                                     krrow[0:1, ch * 512:(ch + 1) * 512],
                                     start=True, stop=True)
                    nc.vector.tensor_copy(kr_b[:, ch * 512:(ch + 1) * 512],
                                          kr_ps[:])

                # ---------- update + z_r + norms, tile by tile ----------
                zr = psm.tile([128, RT], F32, tag="zr")
                npc = psm.tile([128, RT], F32, tag="npc")
                for rt in range(RT):
                    s1 = ps1.tile([128, N_DIM], BF16, tag="s1")
                    nc.vector.tensor_tensor(s1[:], mem[:, msl(rt)], e_b[:],
                                            AL.mult)
                    nc.vector.tensor_tensor(s1[:], a_b[:], s1[:], AL.subtract)
                    nc.vector.scalar_tensor_tensor(
                        mem[:, msl(rt)], s1[:], w_col[:, rt:rt + 1],
                        mem[:, msl(rt)], AL.mult, AL.add)
                    dz2 = psm.tile([128, 1], F32, tag="dz2")
                    nc.vector.scalar_tensor_tensor(
                        dz2[:].broadcast_to([128, N_DIM]), mem[:, msl(rt)],
                        1.0, kr_b[:], AL.mult, AL.mult,
                        accum_out=zr[:, rt:rt + 1])
                    nc.scalar.activation(s1[:], mem[:, msl(rt)], ACT.Square,
                                         accum_out=npc[:, rt:rt + 1])

                # ---------- read logits + local stats ----------
                sqrtn_new = pstate.tile([128, RT], F32, tag="sqrtn")
                nc.scalar.activation(sqrtn_new[:], npc[:], ACT.Sqrt)
                sqrtn = sqrtn_new
                den_r = psm.tile([128, RT], F32, tag="den_r")
                nc.vector.tensor_scalar(den_r[:], sqrtn[:], EPS, None, AL.add)
                rec_r = psm.tile([128, RT], F32, tag="rec_r")
                nc.vector.reciprocal(rec_r[:], den_r[:])
                li_r = psm.tile([128, RT], F32, tag="li_r")
                nc.vector.tensor_tensor(li_r[:], zr[:], rec_r[:], AL.mult)
                lmax_r, lsum_r, u_col = colstats(li_r[:], "r")

                # ---------- AllGather read stats (overlaps rp matmuls) ----
                pay_r = prow.tile([1, 2], F32, tag="pay_a")
                nc.vector.tensor_copy(pay_r[0:1, 0:1], lmax_r[0:1, :])
                nc.vector.tensor_copy(pay_r[0:1, 1:2], lsum_r[0:1, :])
                ag_r_in = pdram.tile([1, 2], F32, tag="ag_r_in")
                ag_r_out = pdram.tile([N_CORES, 2], F32, tag="ag_r_out")
                nc.sync.dma_start(ag_r_in[:], pay_r[:])
                nc.gpsimd.collective_compute(
                    "AllGather", AL.bypass, replica_groups=RG,
                    ins=[ag_r_in.opt()], outs=[ag_r_out.opt()])
                st8r = psm.tile([N_CORES, 2], F32, tag="st8")
                nc.sync.dma_start(st8r[:], ag_r_out[:])

                # u_col (f32) is the local unnormalized read weight; compute
                # partial read rp = sum_r u_r * mem'[r, :] on TensorE.
                ucast = psm.tile([128, RT], MDT, tag="ucast")
                if mem_bf16:
                    nc.vector.tensor_copy(ucast[:], u_col[:])
                    u_lhs = ucast
                else:
                    u_lhs = u_col
                rp_pss = []
                for cc in range(4):
                    rp_ps = ppsb.tile([1, 1024], F32, tag="rp_ps")
                    for half in range(2):
                        base = cc * 1024 + half * 512
                        for rt in range(RT):
                            nc.tensor.matmul(
                                rp_ps[0:1, half * 512:half * 512 + 512],
                                u_lhs[:, rt:rt + 1],
                                mem[:, rt * N_DIM + base:rt * N_DIM + base + 512],
                                start=(rt == 0), stop=(rt == RT - 1))
                    rp_pss.append(rp_ps)

                # combine weight for this core: exp(lmax_r - gmax) / D
                gm8r = psm.tile([N_CORES, 1], F32, tag="gm8")
                nc.gpsimd.partition_all_reduce(gm8r[:], st8r[:, 0:1], N_CORES,
                                               ReduceOp.max)
                ng8r = psm.tile([N_CORES, 1], F32, tag="ng8")
                nc.vector.tensor_scalar(ng8r[:], gm8r[:], -1.0, None, AL.mult)
                eh8r = psm.tile([N_CORES, 1], F32, tag="eh8")
                nc.scalar.activation(eh8r[:], st8r[:, 0:1], ACT.Exp,
                                     bias=ng8r[:])
                pr8r = psm.tile([N_CORES, 1], F32, tag="pr8")
                nc.vector.tensor_tensor(pr8r[:], eh8r[:], st8r[:, 1:2],
                                        AL.mult)
                d8 = psm.tile([N_CORES, 1], F32, tag="gs8")
                nc.gpsimd.partition_all_reduce(d8[:], pr8r[:], N_CORES,
                                               ReduceOp.add)
                grow_r = prow.tile([1, 2], F32, tag="grow")
                nc.vector.tensor_copy(grow_r[0:1, 0:1], gm8r[0:1, :])
                nc.vector.tensor_copy(grow_r[0:1, 1:2], d8[0:1, :])
                # own combine weight on partition 0: exp(lmax_own - gmax)/D
                dinv1 = prow.tile([1, 1], F32, tag="dinv1")
                nc.vector.reciprocal(dinv1[:], grow_r[0:1, 1:2])
                ngm1 = prow.tile([1, 1], F32, tag="ngm1")
                nc.vector.tensor_scalar(ngm1[:], grow_r[0:1, 0:1], -1.0, None,
                                        AL.mult)
                cw1 = prow.tile([1, 1], F32, tag="cw1")
                nc.scalar.activation(cw1[:], lmax_r[0:1, :], ACT.Exp,
                                     bias=ngm1[:])
                nc.vector.tensor_tensor(cw1[:], cw1[:], dinv1[:], AL.mult)

                # scale partial reads (fused into PSUM->SBUF copy), AllReduce
                ar_in = pdram.tile([1, N_DIM], F32, tag="ar_in")
                ar_out = pdram.tile([1, N_DIM], F32, tag="ar_out")
                for cc in range(4):
                    rp_sb = prp.tile([1, 1024], F32, tag="rp_sb")
                    nc.vector.tensor_scalar(rp_sb[:], rp_pss[cc][:],
                                            cw1[0:1, :], None, AL.mult)
                    nc.sync.dma_start(ar_in[0:1, cc * 1024:(cc + 1) * 1024],
                                      rp_sb[:])
                nc.gpsimd.collective_compute(
                    "AllReduce", AL.add, replica_groups=RG,
                    ins=[ar_in.opt()], outs=[ar_out.opt()])

                # ---------- executioner: X <- tanh(X @ R) ----------
                r_col = psm.tile([FVS, FVS], F32, tag="r_col")
                nc.sync.dma_start(
                    r_col[:], ar_out[:].rearrange("one (i j) -> (one i) j", i=FVS))
                x_ps = ppsc.tile([FVS, 1], F32, tag="mini")
                nc.tensor.matmul(x_ps[:], r_col[:], x_col[:],
                                 start=True, stop=True)
                x_new = pstate.tile([FVS, 1], F32, tag="xcol")
                nc.scalar.activation(x_new[:], x_ps[:], ACT.Tanh)
                x_col = x_new

            # ---------- output: Xf @ output_embedding ----------
            o_ps = ppsc.tile([1, NOUT], F32, tag="mini")
            nc.tensor.matmul(o_ps[:], x_col[:], oesb[:], start=True, stop=True)
            o_sb = prow.tile([1, NOUT], F32, tag="o_sb")
            nc.vector.tensor_copy(o_sb[:], o_ps[:])
            nc.sync.dma_start(d_out[:], o_sb[:])

    nc.compile()
    return nc


def host_prep(inputs, mem_bf16=MEM_BF16):
    import ml_dtypes
    bf16 = ml_dtypes.bfloat16
    f32 = np.float32

    x = np.asarray(inputs["x"], f32)
    program = np.asarray(inputs["program"], f32)
    memory0 = np.asarray(inputs["memory0"], f32)
    ie = np.asarray(inputs["input_embedding"], f32)
    oe = np.asarray(inputs["output_embedding"], f32)
    Wc = np.asarray(inputs["Wc"], f32)
    bc = np.asarray(inputs["bc"], f32)
    Wk = np.asarray(inputs["Wk"], f32)
    bk = np.asarray(inputs["bk"], f32)
    We = np.asarray(inputs["We"], f32)
    be = np.asarray(inputs["be"], f32)
    Wa = np.asarray(inputs["Wa"], f32)
    ba = np.asarray(inputs["ba"], f32)
    Wrk = np.asarray(inputs["Wrk"], f32)
    brk = np.asarray(inputs["brk"], f32)

    x0col = (x @ ie).astype(f32).reshape(FVS, 1)

    progpad = np.zeros((128, NSTEPS), f32)
    progpad[FVS:128, :] = program[0].T          # rows 64:128 = prog_t

    wct = np.ascontiguousarray(Wc.T)            # [128, 256]
    bccol = np.ascontiguousarray(bc.reshape(2, 128).T)  # bccol[p,h]=bc[h*128+p]

    wt = np.concatenate([Wk.T, We.T, Wa.T], axis=1).astype(bf16)  # [256,12288]
    wtb = np.concatenate([bk, be, ba]).reshape(1, 3 * N_DIM).astype(bf16)

    kr = np.tanh(program[0] @ Wrk.T + brk)      # [8, 4096]
    kr = kr / np.linalg.norm(kr, axis=1, keepdims=True)
    krall = kr.astype(bf16)

    onesrow = np.ones((1, 128), bf16)

    mdt = bf16 if mem_bf16 else f32
    common = {
        "x0col": x0col, "progpad": progpad, "wct": wct, "bccol": bccol,
        "wt": wt, "wtb": wtb, "krall": krall,
        "oesb": np.ascontiguousarray(oe), "onesrow": onesrow,
    }
    in_maps = []
    for r in range(N_CORES):
        shard = memory0[r * M_LOC:(r + 1) * M_LOC, :]
        n = np.sqrt((shard.astype(np.float64) ** 2).sum(1)).astype(f32)
        sqrtn0 = np.ascontiguousarray(n.reshape(RT, 128).T)  # [p, t]
        m = dict(common)
        m["mem"] = np.ascontiguousarray(
            shard.reshape(RT, 128, N_DIM).transpose(1, 0, 2)
            .reshape(128, RT * N_DIM).astype(mdt))
        m["sqrtn0"] = sqrtn0
        in_maps.append(m)
    return in_maps


def kernel(**inputs):
    from concourse.bass_utils import run_bass_kernel_spmd
    key = ("nc", NSTEPS, MEM_BF16)
    if key not in _CACHE:
        _CACHE[key] = build_nc(NSTEPS, MEM_BF16)
    nc = _CACHE[key]
    in_maps = host_prep(inputs, MEM_BF16)
    res = run_bass_kernel_spmd(nc, in_maps, core_ids=list(range(N_CORES)))
    return np.asarray(res.results[0]["out"], np.float32)

